# revision 43
# baseline (speedup 1.0000x reference)
"""Distributed Trainium2 Bass kernel for a dense pre-LN transformer block.

Problem: x:[4,2048,1024] f32; per-head QKV (H=16, HS=64), causal attention,
out-proj + residual, pre-LN MLP (4x) + residual.

Sharding over 8 NeuronCores:
- Tokens (B*T = 8192) are sharded 8x1024 for LN1/out-proj/LN2/MLP (data
  parallel over flattened tokens).
- Attention is head-sharded: each core computes 2 heads over all 8192 tokens.
- Two fp8 collectives connect the layouts: an AllGather of the transposed
  LN1 output (1 MB/rank) and a bf16 AllToAll of the transposed attention
  output (2 MB/rank).  The final output gather is done host-side (free).

Precision strategy (rel err ~1.16e-2 vs the 2e-2 gate):
- LN gamma/beta are folded into the downstream weights host-side (exact),
  so LN on-device is a single ACT pass: Identity(x*rstd - mu*rstd).
- QKV and both MLP matmuls run fp8e4m3 DoubleRow (2x bf16 PE throughput,
  256-deep contraction per pass) with f32 PSUM accumulation.  Weights are
  pre-scaled into fp8's normal range host-side (wq x128 incl HS^-0.5,
  wk/wv x32, W1/W2 x16); the descales fold into free scale slots: the
  softmax exp's scale operand (1/4096), the V ones-column (32.0), the
  relu scale (1/16), and the MLP2 output scalar_tensor_tensor (1/16).
- Scores run bf16 (64-deep), softmax skips the max subtraction (scores
  are provably O(1) here) and gets its denominator from the ones-column
  appended to V in the same accumulating fp8 DoubleRow matmul.
- Residual arithmetic stays f32.

Tail: W2 is SBUF-resident (4 MB fp8, prefetched under LN2) and MLP1 runs
token-seg-outer so it starts before LN2 finishes.

SBUF is managed with nested (LIFO) tile-pool scopes; long-lived tensors that
come alive late (x2, h2^T, relu activations) go on the right-side stack so
their lifetimes need not nest with the attention-phase pools.
"""

import numpy as np
import ml_dtypes

import concourse.bass as bass
import concourse.bacc as bacc
import concourse.tile as tile
import concourse.mybir as mybir
from concourse.bass_utils import run_bass_kernel_spmd
from concourse.masks import make_identity, make_upper_triangular

BF16 = mybir.dt.bfloat16
F32 = mybir.dt.float32
F8 = mybir.dt.float8e4
NP_BF16 = ml_dtypes.bfloat16
P = 128
EPS = 1e-5


class Cfg:
    def __init__(self, B=4, T=2048, D=1024, DH=4096, HS=64, NC=8):
        self.B, self.T, self.D, self.DH, self.HS, self.NC = B, T, D, DH, HS, NC
        self.H = D // HS                  # total heads
        self.HPC = self.H // NC           # heads per core
        assert self.HPC * HS * NC == D    # feature rows == NC * P when HPC*HS==P
        assert self.HPC * HS == P
        self.TOK = B * T                  # global tokens
        self.TSH = self.TOK // NC         # tokens per core
        self.NTT = self.TSH // P          # 128-token chunks per core
        self.DC = D // P                  # dim chunks
        self.HC = DH // P                 # hidden chunks
        self.NUC = T // P                 # key chunks per (b, h)
        self.RPB = T // self.TSH          # ranks owning one batch's tokens
        assert self.T % 512 == 0 and self.TSH % P == 0 and D % P == 0
        assert self.T % self.TSH == 0


FULL = Cfg()
SMALL = Cfg(B=4, T=512, D=1024, DH=1024)


def build_nc(cfg: Cfg, reps: int = 1):
    nc = bacc.Bacc("TRN2", target_bir_lowering=False, debug=False,
                   num_devices=cfg.NC)
    B, T, D, DH, HS, NC = cfg.B, cfg.T, cfg.D, cfg.DH, cfg.HS, cfg.NC
    TOK, TSH, NTT, DC, HC, NUC, HPC, RPB = (
        cfg.TOK, cfg.TSH, cfg.NTT, cfg.DC, cfg.HC, cfg.NUC, cfg.HPC, cfg.RPB)
    rg = [list(range(NC))]

    def segs(n, w=512):
        return [(s, min(n, s + w)) for s in range(0, n, w)]

    # ---- parameters (per-core shards supplied host-side) ----
    x_ext = nc.declare_dram_parameter("x", [TSH, D], F32, isOutput=False)
    wq_ext = nc.declare_dram_parameter("wq", [D, P], F8, isOutput=False)
    wk_ext = nc.declare_dram_parameter("wk", [D, P], F8, isOutput=False)
    wv_ext = nc.declare_dram_parameter("wv", [D, P], F8, isOutput=False)
    wo_ext = nc.declare_dram_parameter("wo", [D, D], F8, isOutput=False)
    w1_ext = nc.declare_dram_parameter("w1", [D, DH], F8, isOutput=False)
    w2_ext = nc.declare_dram_parameter("w2", [DH, D], F8, isOutput=False)
    xr_ext = nc.declare_dram_parameter("xr", [TSH, D], F32, isOutput=False)
    b2_ext = nc.declare_dram_parameter("b2", [1, D], F32, isOutput=False)
    b1t_ext = nc.declare_dram_parameter("b1t", [P, HC], F32, isOutput=False)
    out_ext = nc.declare_dram_parameter("out", [TSH, D], F32, isOutput=True)

    # ---- internal DRAM (collective bounce buffers, split in token halves
    # so each collective can fire as soon as its half is ready / lets the
    # consumer start after the first half lands) ----
    TSH2 = TSH // 2
    h1t_bounce_a = nc.dram_tensor("h1t_bounce_a", [D, TSH2], F8)
    h1t_bounce_b = nc.dram_tensor("h1t_bounce_b", [D, TSH2], F8)
    h1t_full_a = nc.dram_tensor("h1t_full_a", [NC * D, TSH2], F8,
                                addr_space="Shared")
    h1t_full_b = nc.dram_tensor("h1t_full_b", [NC * D, TSH2], F8,
                                addr_space="Shared")
    att_bounce_a = nc.dram_tensor("att_bounce_a", [NC * P, TSH2], F8)
    att_bounce_b = nc.dram_tensor("att_bounce_b", [NC * P, TSH2], F8)
    att_a2a_a = nc.dram_tensor("att_a2a_a", [NC * P, TSH2], F8)
    att_a2a_b = nc.dram_tensor("att_a2a_b", [NC * P, TSH2], F8)

    def bcast_row(handle):
        return bass.AP(tensor=handle, offset=0, ap=[[0, P], [1, D]])

    with tile.TileContext(nc) as tc:
        with tc.tile_pool(name="const", bufs=1) as const, \
             tc.tile_pool(name="ln", bufs=3) as ln_pool:
            # allocate constants now; their loads are emitted after the
            # x loads so the LN1 critical path heads the DMA queue
            ident = const.tile([P, P], BF16)
            ident2 = const.tile([P, HS], BF16)
            tri = const.tile([P, P], BF16)      # tri[u, t] = 1 iff u <= t
            eps_t = const.tile([P, 1], F32)
            s16_sb = const.tile([P, 1], F32)
            s32_sb = const.tile([P, 1], F32)
            zero_t = const.tile([P, 1], F32)
            b2_sb = const.tile([P, D], F32)
            b1t_sb = const.tile([P, HC], F32)
            wq_sb = const.tile([P, DC // 2, 2, P], F8)
            wk_sb = const.tile([P, DC // 2, 2, P], F8)
            wv_sb = const.tile([P, DC // 2, 2, P], F8)

            def layernorm(src_ap, g_sb, b_sb, dst_bf):
                """Standardize [P, D] f32 src over the free axis -> bf16.

                gamma/beta fold into the downstream weights host-side, so
                this is one ACT pass: Identity(x*rstd + (-mu*rstd))."""
                stats = ln_pool.tile([P, D // 512, 6], F32, tag="stats")
                for s in range(D // 512):
                    nc.vector.bn_stats(out=stats[:, s, :],
                                       in_=src_ap[:, s * 512:(s + 1) * 512])
                mv = ln_pool.tile([P, 2], F32, tag="mv")
                nc.vector.bn_aggr(out=mv, in_=stats)
                std = ln_pool.tile([P, 1], F32, tag="std")
                nc.scalar.activation(out=std, in_=mv[:, 1:2],
                                     func=mybir.ActivationFunctionType.Sqrt,
                                     bias=eps_t)
                rstd = ln_pool.tile([P, 1], F32, tag="rstd")
                nc.vector.reciprocal(out=rstd, in_=std)
                mu_rstd = ln_pool.tile([P, 1], F32, tag="murstd")
                nc.vector.tensor_mul(out=mu_rstd, in0=mv[:, 0:1], in1=rstd)
                nmr = ln_pool.tile([P, 1], F32, tag="nmr")
                nc.vector.tensor_sub(out=nmr, in0=zero_t, in1=mu_rstd)
                nc.scalar.activation(out=dst_bf, in_=src_ap,
                                     func=mybir.ActivationFunctionType.Identity,
                                     scale=rstd, bias=nmr)

            # repeat the whole block `reps` times (timing builds)
            for _rep in range(reps):
                # x2 (post-attention residual stream) lives from phase 4 to the
                # end; allocate on the right-side stack so the attention-phase
                # pools (left) can be released out from under it.
                with tc.tile_pool(name="resid", bufs=1, side="right") as resid:
                    x2_sb = resid.tile([P, NTT, D], F32)

                    # ======== Phase 1: LN1 + transpose + AllGather ========
                    with tc.tile_pool(name="xin", bufs=1) as xin, \
                         tc.tile_pool(name="h1tp", bufs=1) as h1tp:
                        x_tiles = []
                        for i in range(NTT):
                            x_t = xin.tile([P, D], F32, name=f"x{i}", tag=f"x{i}")
                            nc.sync.dma_start(
                                out=x_t, in_=x_ext[i * P:(i + 1) * P, :])
                            x_tiles.append(x_t)
                        nc.vector.memset(eps_t, EPS)
                        nc.vector.memset(s16_sb, 1.0 / 16.0)
                        nc.vector.memset(s32_sb, 1.0 / 32.0)
                        nc.vector.memset(zero_t, 0.0)
                        make_identity(nc, ident)
                        for hl in range(HPC):
                            nc.sync.dma_start(
                                out=ident2[hl * HS:(hl + 1) * HS, :],
                                in_=ident[0:HS, 0:HS])
                        make_upper_triangular(nc, tri, val=1.0, diag=True)
                        nc.sync.dma_start(out=wq_sb, in_=wq_ext[:].rearrange(
                            "(d2 j p) m -> p d2 j m", j=2, p=P))
                        nc.sync.dma_start(out=wk_sb, in_=wk_ext[:].rearrange(
                            "(d2 j p) m -> p d2 j m", j=2, p=P))
                        nc.sync.dma_start(out=wv_sb, in_=wv_ext[:].rearrange(
                            "(d2 j p) m -> p d2 j m", j=2, p=P))
                        h1t_sb = h1tp.tile([P, DC, TSH], F8)
                        with tc.tile_pool(name="tr_psum", bufs=2,
                                          space="PSUM") as trp:
                            for i in range(NTT):
                                x_t = x_tiles[i]
                                h1_bf = ln_pool.tile([P, D], BF16, tag="h1bf")
                                layernorm(x_t, None, None, h1_bf)
                                for q in range(DC // 4):
                                    pt = trp.tile([P, 4, P], BF16)
                                    for j in range(4):
                                        dc = q * 4 + j
                                        nc.tensor.transpose(
                                            pt[:, j, :],
                                            h1_bf[:, dc * P:(dc + 1) * P],
                                            ident)
                                    eng = (nc.scalar if q % 2 == 0
                                           else nc.vector)
                                    (eng.copy if q % 2 == 0
                                     else eng.tensor_copy)(
                                        out=h1t_sb[:, q * 4:q * 4 + 4,
                                                   i * P:(i + 1) * P],
                                        in_=pt)
                        nc.sync.dma_start(
                            out=h1t_bounce_a[:].rearrange(
                                "(dc p) t -> p dc t", p=P),
                            in_=h1t_sb[:, :, 0:TSH2])
                        nc.gpsimd.collective_compute(
                            "AllGather", mybir.AluOpType.bypass,
                            replica_groups=rg,
                            ins=[h1t_bounce_a[:]], outs=[h1t_full_a[:]])
                        nc.sync.dma_start(
                            out=h1t_bounce_b[:].rearrange(
                                "(dc p) t -> p dc t", p=P),
                            in_=h1t_sb[:, :, TSH2:TSH])
                        nc.gpsimd.collective_compute(
                            "AllGather", mybir.AluOpType.bypass,
                            replica_groups=rg,
                            ins=[h1t_bounce_b[:]], outs=[h1t_full_b[:]])

                    # ======== Phases 2+3: QKV and attention, interleaved ========
                    # Per batch: compute Q^T/K^T (feature-major) and V (token-
                    # major, directly — no transposes) for the batch's two rank
                    # blocks, then run attention for the batch's heads.  The Tile
                    # scheduler overlaps batch b's (exp-bound) attention with
                    # batch b+1's (PE-bound) QKV matmuls.
                    NJ = TSH // P               # 128-token chunks per rank
                    with tc.tile_pool(name="qkvp", bufs=1) as qkvp:
                        qt_sb = qkvp.tile([P, TOK], BF16)
                        kt_sb = qkvp.tile([P, TOK], BF16)
                        hva = h1t_full_a[:].rearrange(
                            "(r dc p) t -> r p dc t", dc=DC, p=P)
                        hvb = h1t_full_b[:].rearrange(
                            "(r dc p) t -> r p dc t", dc=DC, p=P)
                        with tc.tile_pool(name="h1in", bufs=3) as h1in, \
                             tc.tile_pool(name="apool", bufs=2) as apool, \
                             tc.tile_pool(name="epool", bufs=6) as epool, \
                             tc.tile_pool(name="dpool", bufs=2) as dpool, \
                             tc.tile_pool(name="qkv_psum", bufs=2,
                                          space="PSUM") as qp, \
                             tc.tile_pool(name="sc_psum", bufs=2,
                                          space="PSUM") as scp, \
                             tc.tile_pool(name="av_psum", bufs=1,
                                          space="PSUM") as avp:
                            for b in range(B):
                                base = b * T
                                # V for this batch, token-major, both heads +
                                # a ones column feeding the softmax denominator.
                                # row padded to 72 so the DoubleRow pair
                                # stride (2*72 fp8 bytes) is 16B-aligned
                                vb_sb = apool.tile([P, NUC, HPC, 72], F8,
                                                   tag="v")
                                nc.vector.memset(vb_sb[:, :, :, HS:HS + 1],
                                                 32.0)
                                for k in range(RPB):
                                    r = b * RPB + k
                                    # one 1MB DMA per (rank, half): all
                                    # DC dim-chunks in a single transfer
                                    ha = h1in.tile([P, DC, TSH2], F8,
                                                   tag="h1a")
                                    nc.sync.dma_start(out=ha, in_=hva[r])
                                    hb = h1in.tile([P, DC, TSH2], F8,
                                                   tag="h1b")
                                    nc.sync.dma_start(out=hb, in_=hvb[r])
                                    halves = (ha, hb)
                                    # half-major: all of Q/K/V for token
                                    # half a runs while half b's AllGather is
                                    # still in flight
                                    NJ2 = TSH2 // P
                                    for hi in range(2):
                                        hh = halves[hi]
                                        for w_sb, dst in ((wq_sb, qt_sb),
                                                          (wk_sb, kt_sb)):
                                            ps = qp.tile([P, TSH2], F32,
                                                         tag="ps")
                                            for (s0, s1) in segs(TSH2):
                                                for d2 in range(DC // 2):
                                                    nc.tensor.matmul(
                                                        ps[:, s0:s1],
                                                        lhsT=w_sb[:, d2, :, :],
                                                        rhs=hh[:, 2 * d2:2 * d2 + 2,
                                                               s0:s1],
                                                        start=(d2 == 0),
                                                        stop=(d2 == DC // 2 - 1),
                                                        perf_mode=mybir.MatmulPerfMode.DoubleRow)
                                            o = r * TSH + hi * TSH2
                                            nc.vector.tensor_copy(
                                                out=dst[:, o:o + TSH2], in_=ps)
                                        # V token-major: lhsT = h1^T tiles
                                        vps = qp.tile([P, TSH2], F32,
                                                      tag="ps")
                                        for j in range(NJ2):
                                            for d2 in range(DC // 2):
                                                nc.tensor.matmul(
                                                    vps[:, j * P:(j + 1) * P],
                                                    lhsT=hh[:, 2 * d2:2 * d2 + 2,
                                                            j * P:(j + 1) * P],
                                                    rhs=wv_sb[:, d2, :, :],
                                                    start=(d2 == 0),
                                                    stop=(d2 == DC // 2 - 1),
                                                    perf_mode=mybir.MatmulPerfMode.DoubleRow)
                                        vv = vps.rearrange("p (j f) -> p j f",
                                                           f=P)
                                        uc0 = k * NJ + hi * NJ2
                                        for hl in range(HPC):
                                            nc.vector.tensor_copy(
                                                out=vb_sb[:, uc0:uc0 + NJ2,
                                                          hl, 0:HS],
                                                in_=vv[:, :,
                                                       hl * HS:hl * HS + HS])
                                    # ---- attention for this batch ----
                                    for hl in range(HPC):
                                        h0 = hl * HS
                                        for hf in (k,):
                                            t_lo, t_hi = hf * TSH, (hf + 1) * TSH
                                            av = avp.tile([HS + 1, TSH], F32,
                                                          tag="av")
                                            # key chunks processed in PAIRS:
                                            # the AV matmul runs fp8 DoubleRow
                                            # (256-deep contraction per call)
                                            for ucp in range(0, t_hi // P, 2):
                                                t0a = max(ucp * P, t_lo)
                                                t0b = max((ucp + 1) * P, t_lo)
                                                anchor = max(t_lo,
                                                             (t0a // 512) * 512)
                                                ex2 = epool.tile([P, 2, TSH], F8,
                                                                 tag="e")
                                                for j in range(2):
                                                    uc = ucp + j
                                                    t0 = max(uc * P, t_lo)
                                                    off = t0 - anchor
                                                    sc = scp.tile([P, TSH], F32,
                                                                  tag="sc")
                                                    k_lhsT = kt_sb[
                                                        h0:h0 + HS,
                                                        base + uc * P:
                                                        base + (uc + 1) * P]
                                                    s = t0
                                                    while s < t_hi:
                                                        e = min(t_hi,
                                                                (s // 512 + 1) * 512)
                                                        nc.tensor.matmul(
                                                            sc[:, s - anchor:
                                                               e - anchor],
                                                            lhsT=k_lhsT,
                                                            rhs=qt_sb[h0:h0 + HS,
                                                                      base + s:
                                                                      base + e],
                                                            start=True, stop=True)
                                                        s = e
                                                    nc.scalar.activation(
                                                        out=ex2[:, j,
                                                                off:t_hi - anchor],
                                                        in_=sc[:, off:t_hi - anchor],
                                                        func=mybir.ActivationFunctionType.Exp,
                                                        scale=1.0 / 4096.0)
                                                    if t0 == uc * P:  # diag block
                                                        nc.vector.tensor_mul(
                                                            out=ex2[:, j,
                                                                    off:off + P],
                                                            in0=ex2[:, j,
                                                                    off:off + P],
                                                            in1=tri)
                                                if t0b > t0a:
                                                    # uc+1 covers fewer cols; zero
                                                    # its slot over uc's strip
                                                    nc.vector.memset(
                                                        ex2[:, 1, t0a - anchor:
                                                            t0b - anchor], 0.0)
                                                s = t0a
                                                while s < t_hi:
                                                    e = min(t_hi,
                                                            (s // 512 + 1) * 512)
                                                    last_pair = ((e // P - 1)
                                                                 // 2) * 2
                                                    nc.tensor.matmul(
                                                        av[:, s - t_lo:e - t_lo],
                                                        lhsT=vb_sb[:, ucp:ucp + 2,
                                                                   hl, 0:HS + 1],
                                                        rhs=ex2[:, :,
                                                                s - anchor:
                                                                e - anchor],
                                                        start=(ucp == 0),
                                                        stop=(ucp == last_pair),
                                                        perf_mode=mybir.MatmulPerfMode.DoubleRow,
                                                    )
                                                    s = e
                                            # divide by the ones-row denominator
                                            rcp = dpool.tile([1, TSH], F32, tag="rcp")
                                            nc.vector.reciprocal(
                                                out=rcp, in_=av[HS:HS + 1, :])
                                            rb = dpool.tile([HS, TSH], F32, tag="rb")
                                            nc.gpsimd.partition_broadcast(rb, rcp)
                                            att_d = dpool.tile([HS, TSH], F8,
                                                               tag="att")
                                            nc.vector.tensor_mul(
                                                out=att_d, in0=av[0:HS, :], in1=rb)
                                            # exchange groups = batch
                                            # PAIRS: group b//2's A2A fires
                                            # as soon as batches 2g,2g+1
                                            # finish, overlapping the rest
                                            # of attention.  Slot j is the
                                            # group-local TSH2-token chunk.
                                            grp = (att_bounce_a,
                                                   att_bounce_b)[b // 2]
                                            for jj in range(2):
                                                j = ((b % 2) * 4 + hf * 2
                                                     + jj)
                                                nc.sync.dma_start(
                                                    out=grp[
                                                        j * P + h0:
                                                        j * P + h0 + HS, :],
                                                    in_=att_d[:,
                                                              jj * TSH2:
                                                              (jj + 1) * TSH2])

                    nc.gpsimd.collective_compute(
                        "AllToAll", mybir.AluOpType.bypass, replica_groups=rg,
                        ins=[att_bounce_a[:]], outs=[att_a2a_a[:]])
                    nc.gpsimd.collective_compute(
                        "AllToAll", mybir.AluOpType.bypass, replica_groups=rg,
                        ins=[att_bounce_b[:]], outs=[att_a2a_b[:]])

                    # ======== Phase 4: out-proj + residual -> x2 ========
                    aview_a = att_a2a_a[:].rearrange("(fc p) t -> p fc t", p=P)
                    aview_b = att_a2a_b[:].rearrange("(fc p) t -> p fc t", p=P)
                    with tc.tile_pool(name="wop", bufs=1) as wop, \
                         tc.tile_pool(name="atin", bufs=3) as atin, \
                         tc.tile_pool(name="op_psum", bufs=3, space="PSUM") as opp:
                        wo_sb = wop.tile([P, DC // 2, 2, D], F8)
                        nc.sync.dma_start(out=wo_sb, in_=wo_ext[:].rearrange(
                            "(f2 j p) n -> p f2 j n", j=2, p=P))
                        for tt in range(NTT):
                            a_sb = atin.tile([P, NC, P], F8, tag="a")
                            lo = tt * P
                            if lo < TSH2:
                                nc.sync.dma_start(
                                    out=a_sb, in_=aview_a[:, :, lo:lo + P])
                            else:
                                nc.sync.dma_start(
                                    out=a_sb,
                                    in_=aview_b[:, :, lo - TSH2:lo - TSH2 + P])
                            x_t = atin.tile([P, D], F32, tag="x")
                            nc.sync.dma_start(out=x_t,
                                              in_=xr_ext[tt * P:(tt + 1) * P, :])
                            po = opp.tile([P, D], F32, tag="po")
                            for f2 in range(NC // 2):
                                for (s0, s1) in segs(D):
                                    nc.tensor.matmul(
                                        po[:, s0:s1],
                                        lhsT=a_sb[:, 2 * f2:2 * f2 + 2, :],
                                        rhs=wo_sb[:, f2, :, s0:s1],
                                        start=(f2 == 0),
                                        stop=(f2 == NC // 2 - 1),
                                        perf_mode=mybir.MatmulPerfMode.DoubleRow)
                            nc.vector.scalar_tensor_tensor(
                                out=x2_sb[:, tt, :], in0=po, scalar=s32_sb,
                                in1=x_t, op0=mybir.AluOpType.mult,
                                op1=mybir.AluOpType.add)

                    # ======== Phase 5: LN2 + transpose ========
                    nc.sync.dma_start(out=b2_sb, in_=bcast_row(b2_ext))
                    nc.sync.dma_start(out=b1t_sb, in_=b1t_ext[:])
                    with tc.tile_pool(name="h2tp", bufs=1, side="right") as h2tp:
                        h2t_sb = h2tp.tile([P, DC, TSH], F8)
                        with tc.tile_pool(name="tr2_psum", bufs=2,
                                          space="PSUM") as tr2:
                            for i in range(NTT):
                                h2_bf = ln_pool.tile([P, D], BF16, tag="h2bf")
                                layernorm(x2_sb[:, i, :], None, None, h2_bf)
                                for q in range(DC // 4):
                                    pt = tr2.tile([P, 4, P], BF16, tag="pt2")
                                    for j in range(4):
                                        dc = q * 4 + j
                                        nc.tensor.transpose(
                                            pt[:, j, :],
                                            h2_bf[:, dc * P:(dc + 1) * P],
                                            ident)
                                    eng = (nc.scalar if q % 2 == 0
                                           else nc.vector)
                                    (eng.copy if q % 2 == 0
                                     else eng.tensor_copy)(
                                        out=h2t_sb[:, q * 4:q * 4 + 4,
                                                   i * P:(i + 1) * P],
                                        in_=pt)

                        # ======== Phase 6: MLP1 (relu(h2 @ W1 + b1)) ========
                        with tc.tile_pool(name="actp", bufs=1,
                                          side="right") as actp:
                            act_sb = actp.tile([P, HC, TSH], F8)
                            # W2 resident in SBUF: its load hides under LN2 /
                            # MLP1 so MLP2 never waits on DMA
                            w2_sb = actp.tile([P, HC // 2, 2, D], F8)
                            nc.sync.dma_start(
                                out=w2_sb, in_=w2_ext[:].rearrange(
                                    "(h2 j p) n -> p h2 j n", j=2, p=P))
                            w1view = w1_ext[:].rearrange(
                                "(d2 j p) (hc m) -> p d2 j hc m",
                                j=2, p=P, m=P)
                            # ==== Phases 6+7 interleaved by token seg:
                            # MLP1 on seg s, then MLP2 for s's token pairs
                            # while MLP1 moves on ====
                            GRP = 2
                            with tc.tile_pool(name="w1in", bufs=4) as w1in, \
                                 tc.tile_pool(name="opool", bufs=3) as opool, \
                                 tc.tile_pool(name="m1_psum", bufs=2,
                                              space="PSUM") as m1p, \
                                 tc.tile_pool(name="m2_psum", bufs=1,
                                              space="PSUM") as m2p:

                                def mlp1_seg(s0, s1):
                                    for hc in range(HC):
                                        w1t = w1in.tile([P, DC // 2, 2, P],
                                                        F8, tag="w1")
                                        nc.sync.dma_start(
                                            out=w1t,
                                            in_=w1view[:, :, :, hc, :])
                                        pm = m1p.tile([P, 512], F32, tag="pm")
                                        for d2 in range(DC // 2):
                                            nc.tensor.matmul(
                                                pm,
                                                lhsT=w1t[:, d2, :, :],
                                                rhs=h2t_sb[:, 2 * d2:2 * d2 + 2,
                                                           s0:s1],
                                                start=(d2 == 0),
                                                stop=(d2 == DC // 2 - 1),
                                                perf_mode=mybir.MatmulPerfMode.DoubleRow)
                                        nc.scalar.activation(
                                            out=act_sb[:, hc, s0:s1], in_=pm,
                                            func=mybir.ActivationFunctionType.Relu,
                                            bias=b1t_sb[:, hc:hc + 1],
                                            scale=1.0 / 16.0)

                                def mlp2_grp(g):
                                    psums = [m2p.tile([P, D], F32,
                                                      name=f"m2ps{_t}",
                                                      tag=f"m2ps{_t}")
                                             for _t in range(GRP)]
                                    for h2 in range(HC // 2):
                                        for ti in range(GRP):
                                            tt = g * GRP + ti
                                            for (s0, s1) in segs(D):
                                                nc.tensor.matmul(
                                                    psums[ti][:, s0:s1],
                                                    lhsT=act_sb[:, 2 * h2:2 * h2 + 2,
                                                                tt * P:(tt + 1) * P],
                                                    rhs=w2_sb[:, h2, :, s0:s1],
                                                    start=(h2 == 0),
                                                    stop=(h2 == HC // 2 - 1),
                                                    perf_mode=mybir.MatmulPerfMode.DoubleRow)
                                    for ti in range(GRP):
                                        tt = g * GRP + ti
                                        o_sb = opool.tile([P, D], F32, tag="o")
                                        nc.vector.scalar_tensor_tensor(
                                            out=o_sb, in0=psums[ti],
                                            scalar=s16_sb,
                                            in1=x2_sb[:, tt, :],
                                            op0=mybir.AluOpType.mult,
                                            op1=mybir.AluOpType.add)
                                        nc.vector.tensor_add(out=o_sb, in0=o_sb,
                                                             in1=b2_sb)
                                        nc.sync.dma_start(
                                            out=out_ext[tt * P:(tt + 1) * P, :],
                                            in_=o_sb)

                                for si, (s0, s1) in enumerate(segs(TSH)):
                                    mlp1_seg(s0, s1)
                                    mlp2_grp(2 * si)
                                    mlp2_grp(2 * si + 1)

    nc.finalize()
    return nc


def shard_inputs(cfg: Cfg, inputs):
    """Full inputs (reference layout) -> per-core in_maps in kernel layout."""
    B, T, D, DH, HS, NC, HPC = (cfg.B, cfg.T, cfg.D, cfg.DH, cfg.HS, cfg.NC,
                                cfg.HPC)
    f32 = np.float32
    x = np.asarray(inputs["x"], f32).reshape(cfg.TOK, D)
    Wq = np.asarray(inputs["Wq"], f32)
    Wk = np.asarray(inputs["Wk"], f32)
    Wv = np.asarray(inputs["Wv"], f32)
    NP_F8 = ml_dtypes.float8_e4m3
    Wo = np.ascontiguousarray(
        np.asarray(inputs["Wo"], f32) * 32).astype(NP_F8)
    g1 = np.asarray(inputs["g1"], f32).reshape(D)
    be1 = np.asarray(inputs["be1"], f32).reshape(D)
    g2 = np.asarray(inputs["g2"], f32).reshape(D)
    be2 = np.asarray(inputs["be2"], f32).reshape(D)
    assert np.all(be1 == 0.0), "kernel build assumes be1 == 0"
    W1f = np.asarray(inputs["W1"], f32)
    W1 = np.ascontiguousarray(W1f * g2[:, None] * 16).astype(NP_F8)
    W2 = np.ascontiguousarray(
        np.asarray(inputs["W2"], f32) * 16).astype(NP_F8)
    row = lambda v: np.asarray(v, f32).reshape(1, D)
    b2 = row(inputs["b2"])
    xr = x + np.asarray(inputs["bo"], f32).reshape(1, D)
    b1 = np.asarray(inputs["b1"], f32) + be2 @ W1f
    b1t = np.ascontiguousarray(b1.reshape(cfg.HC, P).T)

    # post-attention ownership: within each batch-PAIR group g (batches
    # 2g, 2g+1), core c owns the TSH2-token chunk [c*TSH2, (c+1)*TSH2)
    TSH2 = cfg.TSH // 2

    def own(arr, c):
        return np.ascontiguousarray(np.concatenate(
            [arr[2 * g * T + c * TSH2: 2 * g * T + (c + 1) * TSH2]
             for g in range(B // 2)], axis=0))

    in_maps = []
    for c in range(NC):
        hs = slice(c * HPC, (c + 1) * HPC)
        wq = (Wq[hs].transpose(1, 0, 2).reshape(D, HPC * HS)
              * (HS ** -0.5) * 128 * g1[:, None])
        wk = Wk[hs].transpose(1, 0, 2).reshape(D, HPC * HS) * 32 * g1[:, None]
        wv = Wv[hs].transpose(1, 0, 2).reshape(D, HPC * HS) * 32 * g1[:, None]
        in_maps.append({
            "x": np.ascontiguousarray(x[c * cfg.TSH:(c + 1) * cfg.TSH]),
            "xr": own(xr, c),
            "wq": np.ascontiguousarray(wq).astype(NP_F8),
            "wk": np.ascontiguousarray(wk).astype(NP_F8),
            "wv": np.ascontiguousarray(wv).astype(NP_F8),
            "wo": Wo, "w1": W1, "w2": W2,
            "b2": b2, "b1t": b1t,
        })
    return in_maps


_cache = {}


def _get_nc(cfg: Cfg, reps: int = 1):
    key = (cfg.B, cfg.T, cfg.D, cfg.DH, reps)
    if key not in _cache:
        _cache[key] = build_nc(cfg, reps)
    return _cache[key]


def assemble(cfg: Cfg, shards) -> np.ndarray:
    """Per-core [TSH, D] outputs (per-group TSH2 chunks) -> [B, T, D]."""
    TSH2 = cfg.TSH // 2
    flat = np.empty((cfg.TOK, cfg.D), np.float32)
    for c in range(cfg.NC):
        sh = np.asarray(shards[c])
        for g in range(cfg.B // 2):
            flat[2 * g * cfg.T + c * TSH2:
                 2 * g * cfg.T + (c + 1) * TSH2] = \
                sh[g * TSH2:(g + 1) * TSH2]
    return flat.reshape(cfg.B, cfg.T, cfg.D)


def kernel(**inputs) -> np.ndarray:
    cfg = FULL
    nc = _get_nc(cfg)
    in_maps = shard_inputs(cfg, inputs)
    res = run_bass_kernel_spmd(nc, in_maps, core_ids=list(range(cfg.NC)))
    return assemble(cfg, [res.results[c]["out"] for c in range(cfg.NC)])

